# revision 1
# baseline (speedup 1.0000x reference)
"""Single-head attention (B=8, T=2048, C=512, d_k=64) on 8 Trainium2 cores.

Strategy: data-parallel over batch B — one batch element per NeuronCore,
no collectives. Per core:
  1. x tiles [128,512] DMA'd in natural layout, PE-transposed (identity
     matmul) into x^T [c,t] in SBUF (projections contract over c, which
     must sit on partitions).
  2. Q^T,K^T [64,2048] via W-as-weights matmuls; V [t,64] natural via
     x^T-as-weights; a ones-column is appended to V so the attention
     denominator falls out of the AV matmul for free.
  3. Per key-tile j: S^T = (K^T_j)^T Q^T -> PSUM [128,2048]; one ACT exp
     (scale=1/sqrt(64), no max-subtraction needed: scores ~ N(0,1));
     AV: out^T += V'_j^T @ P^T_j accumulated in PSUM over all j.
  4. Epilogue: PE-transpose out^T tiles back to [t,65], divide by the
     denominator column, DMA out.
"""

import numpy as np
from contextlib import ExitStack

import concourse.bass as bass
import concourse.tile as tile
from concourse import bacc
from concourse import mybir
from concourse.bass_utils import run_bass_kernel_spmd
from concourse.masks import make_identity

B, T, C, DK = 8, 2048, 512, 64
N_CORES = 8
FP32 = mybir.dt.float32
P = 128
TT = T // P      # 16 token tiles
CCH = C // P     # 4 contraction chunks
NB = 512         # matmul moving-operand max (fp32)
IC = T // NB     # 4 i-chunks
SCALE = 1.0 / np.sqrt(np.float32(DK))

_cached = {}


def _build_nc():
    nc = bacc.Bacc("TRN2", target_bir_lowering=False, debug=False)
    x_d = nc.declare_dram_parameter("x", [T, C], FP32, isOutput=False)
    wq_d = nc.declare_dram_parameter("Wq", [C, DK], FP32, isOutput=False)
    wk_d = nc.declare_dram_parameter("Wk", [C, DK], FP32, isOutput=False)
    wv_d = nc.declare_dram_parameter("Wv", [C, DK], FP32, isOutput=False)
    out_d = nc.declare_dram_parameter("out", [T, DK], FP32, isOutput=True)

    x_t = x_d.rearrange("(tt p) c -> tt p c", p=P)          # [16,128,512]
    out_t = out_d.rearrange("(tt p) d -> tt p d", p=P)      # [16,128,64]

    with ExitStack() as ctx:
        tc = ctx.enter_context(tile.TileContext(nc))
        const = ctx.enter_context(tc.tile_pool(name="const", bufs=1))

        identity = const.tile([P, P], FP32)
        make_identity(nc, identity)

        # --- weights to SBUF, chunked over c ---
        wq_s = const.tile([P, CCH, DK], FP32)
        wk_s = const.tile([P, CCH, DK], FP32)
        wv_s = const.tile([P, CCH, DK], FP32)
        nc.sync.dma_start(out=wq_s, in_=wq_d.rearrange("(ch p) d -> p ch d", p=P))
        nc.sync.dma_start(out=wk_s, in_=wk_d.rearrange("(ch p) d -> p ch d", p=P))
        nc.sync.dma_start(out=wv_s, in_=wv_d.rearrange("(ch p) d -> p ch d", p=P))

        xT = const.tile([P, CCH, T], FP32)          # x^T, 32KB/part
        v_s = const.tile([P, TT, DK + 1], FP32)     # V with ones col
        nc.vector.memset(v_s, 1.0)
        qT = const.tile([DK, T], FP32)
        kT = const.tile([DK, T], FP32)

        # --- phase 1: load x, transpose into xT; V per t-tile ---
        with (
            tc.tile_pool(name="xload", bufs=3) as xload,
            tc.tile_pool(name="tpsum", bufs=2, space="PSUM") as tpsum,
            tc.tile_pool(name="vpsum", bufs=2, space="PSUM") as vpsum,
        ):
            for tt in range(TT):
                x_tile = xload.tile([P, C], FP32, tag="x_tile")
                nc.sync.dma_start(out=x_tile, in_=x_t[tt])
                for ch in range(CCH):
                    ps = tpsum.tile([P, P], FP32, tag="tps")
                    nc.tensor.transpose(ps, x_tile[:, ch * P:(ch + 1) * P], identity)
                    nc.vector.tensor_copy(out=xT[:, ch, tt * P:(tt + 1) * P], in_=ps)
                pv = vpsum.tile([P, DK], FP32, tag="pv")
                for ch in range(CCH):
                    nc.tensor.matmul(
                        pv, lhsT=xT[:, ch, tt * P:(tt + 1) * P], rhs=wv_s[:, ch, :],
                        start=(ch == 0), stop=(ch == CCH - 1))
                nc.vector.tensor_copy(out=v_s[:, tt, 0:DK], in_=pv)

            # --- phase 2: Q^T, K^T projections ---
            for ic in range(IC):
                pq = vpsum.tile([DK, NB], FP32, tag="pq")
                pk = vpsum.tile([DK, NB], FP32, tag="pk")
                for ch in range(CCH):
                    nc.tensor.matmul(
                        pq, lhsT=wq_s[:, ch, :], rhs=xT[:, ch, ic * NB:(ic + 1) * NB],
                        start=(ch == 0), stop=(ch == CCH - 1))
                for ch in range(CCH):
                    nc.tensor.matmul(
                        pk, lhsT=wk_s[:, ch, :], rhs=xT[:, ch, ic * NB:(ic + 1) * NB],
                        start=(ch == 0), stop=(ch == CCH - 1))
                nc.vector.tensor_copy(out=qT[:, ic * NB:(ic + 1) * NB], in_=pq)
                nc.vector.tensor_copy(out=kT[:, ic * NB:(ic + 1) * NB], in_=pk)

        # --- main loop: S^T -> exp -> AV accumulate ---
        with (
            tc.tile_pool(name="spsum", bufs=1, space="PSUM") as spsum,
            tc.tile_pool(name="opsum", bufs=1, space="PSUM") as opsum,
            tc.tile_pool(name="ppool", bufs=2) as ppool,
        ):
            o_ps = []
            for ic in range(IC):
                o_tile = opsum.tile([DK + 1, NB], FP32, tag=f"ops{ic}")
                o_ps.append(o_tile)
            for j in range(TT):
                pT = ppool.tile([P, T], FP32, tag="pT")
                for h in range(2):
                    s_ps = spsum.tile([P, T // 2], FP32, tag="sps", bufs=2)
                    for ic in range(2):
                        icg = h * 2 + ic
                        nc.tensor.matmul(
                            s_ps[:, ic * NB:(ic + 1) * NB],
                            lhsT=kT[:, j * P:(j + 1) * P],
                            rhs=qT[:, icg * NB:(icg + 1) * NB],
                            start=True, stop=True)
                    nc.scalar.activation(
                        out=pT[:, h * (T // 2):(h + 1) * (T // 2)], in_=s_ps,
                        func=mybir.ActivationFunctionType.Exp, scale=float(SCALE))
                for ic in range(IC):
                    nc.tensor.matmul(
                        o_ps[ic], lhsT=v_s[:, j, :], rhs=pT[:, ic * NB:(ic + 1) * NB],
                        start=(j == 0), stop=(j == TT - 1), skip_group_check=True)

            # --- epilogue: transpose out^T back, normalize, store ---
            oT_s = ppool.tile([DK + 1, T], FP32, tag="oTs", bufs=1)
            for ic in range(IC):
                nc.vector.tensor_copy(out=oT_s[:, ic * NB:(ic + 1) * NB], in_=o_ps[ic])

        with (
            tc.tile_pool(name="epsum", bufs=2, space="PSUM") as epsum,
            tc.tile_pool(name="outp", bufs=3) as outp,
        ):
            for tt in range(TT):
                ot_ps = epsum.tile([P, DK + 1], FP32, tag="otps")
                nc.tensor.transpose(
                    ot_ps, oT_s[:, tt * P:(tt + 1) * P], identity[0:DK + 1, 0:DK + 1])
                recip = outp.tile([P, 1], FP32, tag="recip")
                nc.vector.reciprocal(recip, ot_ps[:, DK:DK + 1])
                o_tile2 = outp.tile([P, DK], FP32, tag="otile")
                nc.vector.tensor_scalar_mul(o_tile2, ot_ps[:, 0:DK], recip)
                nc.sync.dma_start(out=out_t[tt], in_=o_tile2)

    nc.compile()
    return nc


def _get_nc():
    if "nc" not in _cached:
        _cached["nc"] = _build_nc()
    return _cached["nc"]


def kernel(x, Wq, Wk, Wv, **run_kwargs):
    x = np.asarray(x, dtype=np.float32)
    Wq = np.asarray(Wq, dtype=np.float32)
    Wk = np.asarray(Wk, dtype=np.float32)
    Wv = np.asarray(Wv, dtype=np.float32)
    nc = _get_nc()
    in_maps = [
        {"x": np.ascontiguousarray(x[b]), "Wq": Wq, "Wk": Wk, "Wv": Wv}
        for b in range(B)
    ]
    res = run_bass_kernel_spmd(nc, in_maps, list(range(N_CORES)), **run_kwargs)
    out = np.stack([res.results[b]["out"] for b in range(B)], axis=0)
    if run_kwargs:
        _cached["last_result"] = res
    return out



# revision 8
# speedup vs baseline: 2.2828x; 2.2828x over previous
"""Single-head attention (B=8, T=2048, C=512, d_k=64) on 8 Trainium2 cores.

Data-parallel over batch B - one batch element per NeuronCore, no collectives.

v2 design (vs v1 all-fp32 baseline at 208us):
  - Big matmuls run as float32r (fp32 bits, 1 cycle/row at N>=512 vs 4 for
    fp32). Operand tiles are declared float32r so their producers (DVE
    copies / ACT exp) round on write, which the BIR verifier requires.
    PE transposes stay plain fp32 (2 cycles/row, minor).
  - S^T matmuls contract over d_k=64 (half the PE array). Q^T/K^T are
    produced DUPLICATED on both partition halves by col-tiled dual-output
    projections (out partitions 0-63 and 64-127, concurrent in the array),
    so S matmuls for key-tile pairs (2j, 2j+1) run 2-way row-packed
    (tile_position rows 0-63 / 64-127, concurrent).
  - V computed as V^T (N=512 projections) then PE-transposed per key tile;
    ones column appended so the softmax denominator falls out of the AV
    accumulation for free.
  - Main loop is software-pipelined (S of step i+1 emitted before AV of
    step i) so the serial exp chain on ScalarE - the critical resource at
    ~37us - never waits on PE.
  - Emission interleaves tile loads / projections with the first half of
    the main loop so exp starts ~8us in instead of after all projections.
"""

import numpy as np
from contextlib import ExitStack

import concourse.bass as bass
import concourse.tile as tile
from concourse import bacc
from concourse import mybir
from concourse.bass_utils import run_bass_kernel_spmd
from concourse.masks import make_identity

B, T, C, DK = 8, 2048, 512, 64
N_CORES = 8
FP32 = mybir.dt.float32
R = mybir.dt.float32r
P = 128
TT = T // P      # 16 token tiles
CCH = C // P     # 4 contraction chunks
NB = 512         # matmul moving-operand max (4-byte dtypes)
IC = T // NB     # 4 token 512-chunks
SCALE = 1.0 / np.sqrt(np.float32(DK))

_cached = {}


def _build_nc():
    nc = bacc.Bacc("TRN2", target_bir_lowering=False, debug=False)
    x_d = nc.declare_dram_parameter("x", [T, C], FP32, isOutput=False)
    wq_d = nc.declare_dram_parameter("Wq", [C, DK], FP32, isOutput=False)
    wk_d = nc.declare_dram_parameter("Wk", [C, DK], FP32, isOutput=False)
    wv_d = nc.declare_dram_parameter("Wv", [C, DK], FP32, isOutput=False)
    out_d = nc.declare_dram_parameter("out", [T, DK], FP32, isOutput=True)

    x_t = x_d.rearrange("(tt p) c -> tt p c", p=P)          # [16,128,512]
    out_t = out_d.rearrange("(tt p) d -> tt p d", p=P)      # [16,128,64]

    with ExitStack() as ctx:
        tc = ctx.enter_context(tile.TileContext(nc))
        const = ctx.enter_context(tc.tile_pool(name="const", bufs=1))
        xload = ctx.enter_context(tc.tile_pool(name="xload", bufs=3))
        ppool = ctx.enter_context(tc.tile_pool(name="ppool", bufs=3))
        outp = ctx.enter_context(tc.tile_pool(name="outp", bufs=4))
        spool = ctx.enter_context(tc.tile_pool(name="spool", bufs=2, space="PSUM"))
        opool = ctx.enter_context(tc.tile_pool(name="opool", bufs=1, space="PSUM"))
        wpool = ctx.enter_context(tc.tile_pool(name="wpool", bufs=2, space="PSUM"))

        identity = const.tile([P, P], FP32)
        make_identity(nc, identity)

        # warm the exp table set early so the ~2.7us ACT_TABLE_LOAD hides
        # under the load phase
        dum_i = const.tile([P, 1], FP32, name="dumi")
        dum_o = const.tile([P, 1], FP32, name="dumo")
        nc.vector.memset(dum_i, 0.0)
        nc.scalar.activation(out=dum_o, in_=dum_i,
                             func=mybir.ActivationFunctionType.Exp)
        nc.vector.tensor_copy(out=dum_i, in_=dum_o)

        wq_s = const.tile([P, CCH, DK], FP32)
        wk_s = const.tile([P, CCH, DK], FP32)
        wv_s = const.tile([P, CCH, DK], FP32)
        nc.sync.dma_start(out=wq_s, in_=wq_d.rearrange("(ch p) d -> p ch d", p=P))
        nc.sync.dma_start(out=wk_s, in_=wk_d.rearrange("(ch p) d -> p ch d", p=P))
        nc.sync.dma_start(out=wv_s, in_=wv_d.rearrange("(ch p) d -> p ch d", p=P))
        # round weights to f32r; wq/wk doubled along the stationary free dim
        # so one matmul emits Q^T/K^T on BOTH partition halves of the output
        wq_r = const.tile([P, CCH, P], R)
        wk_r = const.tile([P, CCH, P], R)
        wv_r = const.tile([P, CCH, DK], R)
        nc.vector.tensor_copy(out=wq_r[:, :, 0:DK], in_=wq_s)
        nc.vector.tensor_copy(out=wq_r[:, :, DK:P], in_=wq_s)
        nc.vector.tensor_copy(out=wk_r[:, :, 0:DK], in_=wk_s)
        nc.vector.tensor_copy(out=wk_r[:, :, DK:P], in_=wk_s)
        nc.vector.tensor_copy(out=wv_r, in_=wv_s)

        xT = const.tile([P, CCH, T], R)         # x^T chunks, 32KB/part
        # bf16: the 2-way row-packed S matmuls need tile_position rows 64-127,
        # which the ISA only supports for <=2-byte dtypes
        qT2 = const.tile([P, T], mybir.dt.bfloat16)  # Q^T dup on both halves
        kT2 = const.tile([P, T], mybir.dt.bfloat16)  # K^T dup on both halves
        vTs = const.tile([DK, T], FP32)         # V^T
        v_s = const.tile([P, TT, DK + 1], R)    # V with ones col
        ones = const.tile([P, TT], FP32, name="ones")
        nc.vector.memset(ones, 1.0)
        nc.vector.tensor_copy(out=v_s[:, :, DK], in_=ones)
        oT = const.tile([DK + 1, T], FP32)      # out^T staging

        def tile_load(tt):
            x_tile = xload.tile([P, C], FP32, tag="x_tile")
            nc.sync.dma_start(out=x_tile, in_=x_t[tt])
            tps = wpool.tile([P, NB], FP32, tag="wps")
            for ch in range(CCH):
                sl = slice(ch * P, (ch + 1) * P)
                nc.tensor.transpose(tps[:, sl], x_tile[:, sl], identity)
            nc.vector.tensor_copy(
                out=xT[:, :, tt * P:(tt + 1) * P],
                in_=tps[:, :].rearrange("p (ch t) -> p ch t", ch=CCH))

        def proj(ic):
            sl = slice(ic * NB, (ic + 1) * NB)
            pq = wpool.tile([P, NB], FP32, tag="wps")
            for ch in range(CCH):
                nc.tensor.matmul(pq, lhsT=wq_r[:, ch, :], rhs=xT[:, ch, sl],
                                 start=(ch == 0), stop=(ch == CCH - 1))
            nc.vector.tensor_copy(out=qT2[:, sl], in_=pq)
            pk = wpool.tile([P, NB], FP32, tag="wps")
            for ch in range(CCH):
                nc.tensor.matmul(pk, lhsT=wk_r[:, ch, :], rhs=xT[:, ch, sl],
                                 start=(ch == 0), stop=(ch == CCH - 1))
            nc.vector.tensor_copy(out=kT2[:, sl], in_=pk)
            pv = wpool.tile([P, NB], FP32, tag="wps")
            for ch in range(CCH):
                nc.tensor.matmul(pv[0:DK, :], lhsT=wv_r[:, ch, :],
                                 rhs=xT[:, ch, sl],
                                 start=(ch == 0), stop=(ch == CCH - 1))
            nc.vector.tensor_copy(out=vTs[:, sl], in_=pv[0:DK, :])

        def vtrans(j):
            vps = wpool.tile([P, NB], FP32, tag="wps")
            nc.tensor.transpose(
                vps[:, 0:DK], vTs[:, j * P:(j + 1) * P], identity[0:DK, 0:DK])
            nc.vector.tensor_copy(out=v_s[:, j, 0:DK], in_=vps[:, 0:DK])

        # ---- main loop: software-pipelined S -> exp -> AV over 32 steps ----
        # step = (half, jj, qc): key pair (2jj, 2jj+1) x query 512-chunk
        steps = [(h, jj, qc) for h in range(2) for jj in range(TT // 2)
                 for qc in range(2)]

        def emit_S(h, jj, qc):
            s = spool.tile([P, 2 * NB], FP32, tag="sps")
            q0 = h * 1024 + qc * NB
            ja = slice(2 * jj * P, (2 * jj + 1) * P)
            jb = slice((2 * jj + 1) * P, (2 * jj + 2) * P)
            nc.tensor.matmul(s[:, 0:NB], lhsT=kT2[0:DK, ja],
                             rhs=qT2[0:DK, q0:q0 + NB],
                             start=True, stop=True)
            nc.tensor.matmul(s[:, NB:2 * NB], lhsT=kT2[DK:P, jb],
                             rhs=qT2[DK:P, q0:q0 + NB],
                             start=True, stop=True)
            return s

        o_ps = {}

        def emit_tail(i):
            h, jj, qc = steps[i]
            if (jj, qc) == (0, 0):
                o_ps[h] = opool.tile([DK + 1, 2 * NB], FP32, tag="ops",
                                     name=f"ops{h}")
            pT = ppool.tile([P, 2 * NB], R, tag="pT")
            nc.scalar.activation(out=pT, in_=s_tiles[i],
                                 func=mybir.ActivationFunctionType.Exp,
                                 scale=float(SCALE))
            osl = o_ps[h][:, qc * NB:(qc + 1) * NB]
            nc.tensor.matmul(osl, lhsT=v_s[:, 2 * jj, :],
                             rhs=pT[:, 0:NB],
                             start=(jj == 0), stop=False, skip_group_check=True)
            nc.tensor.matmul(osl, lhsT=v_s[:, 2 * jj + 1, :],
                             rhs=pT[:, NB:2 * NB],
                             start=False, stop=(jj == TT // 2 - 1),
                             skip_group_check=True)
            if (jj, qc) == (TT // 2 - 1, 1):
                nc.vector.tensor_copy(
                    out=oT[:, h * 1024:(h + 1) * 1024], in_=o_ps[h])

        def epilogue(tt):
            eps = wpool.tile([P, NB], FP32, tag="wps")
            nc.tensor.transpose(
                eps[:, 0:DK + 1], oT[:, tt * P:(tt + 1) * P],
                identity[0:DK + 1, 0:DK + 1])
            rc = outp.tile([P, 1], FP32, tag="rc", bufs=2)
            nc.vector.reciprocal(rc, eps[:, DK:DK + 1])
            ot = outp.tile([P, DK], FP32, tag="ot")
            nc.vector.tensor_scalar_mul(ot, eps[:, 0:DK], rc)
            nc.sync.dma_start(out=out_t[tt], in_=ot)

        # ---- interleaved emission ----
        for tt in range(8):
            tile_load(tt)
        proj(0)
        proj(1)
        for j in range(8):
            vtrans(j)

        # fillers keyed by step index: emitted between main-loop steps so
        # their priority trails the already-runnable critical path
        fillers = {
            0: lambda: [tile_load(8), tile_load(9)],
            2: lambda: [tile_load(10), tile_load(11)],
            4: lambda: [tile_load(12), tile_load(13), proj(2)],
            6: lambda: [tile_load(14), tile_load(15), proj(3)],
            8: lambda: [vtrans(j) for j in range(8, 16)],
        }

        s_tiles = {}
        s_tiles[0] = emit_S(*steps[0])
        for i in range(len(steps)):
            if i in fillers:
                fillers[i]()
            if i + 1 < len(steps):
                s_tiles[i + 1] = emit_S(*steps[i + 1])
            emit_tail(i)
            del s_tiles[i]

        for tt in range(TT):
            epilogue(tt)

    nc.compile()
    return nc


def _get_nc():
    if "nc" not in _cached:
        _cached["nc"] = _build_nc()
    return _cached["nc"]


def kernel(x, Wq, Wk, Wv, **run_kwargs):
    x = np.asarray(x, dtype=np.float32)
    Wq = np.asarray(Wq, dtype=np.float32)
    Wk = np.asarray(Wk, dtype=np.float32)
    Wv = np.asarray(Wv, dtype=np.float32)
    nc = _get_nc()
    in_maps = [
        {"x": np.ascontiguousarray(x[b]), "Wq": Wq, "Wk": Wk, "Wv": Wv}
        for b in range(B)
    ]
    res = run_bass_kernel_spmd(nc, in_maps, list(range(N_CORES)), **run_kwargs)
    out = np.stack([res.results[b]["out"] for b in range(B)], axis=0)
    if run_kwargs:
        _cached["last_result"] = res
    return out
